# revision 27
# baseline (speedup 1.0000x reference)
"""MoE dense kernel (nn_MoEDense) for 8 Trainium2 NeuronCores.

Reference computation (per batch sample b):
    pooled[b]  = mean_{h,w} x[b,h,w,:]                      # [C_IN]
    logits[b]  = pooled[b] @ gate_W + gate_b                # [E]
    e_b        = argmax(logits[b])       (softmax is monotonic -> skip)
    out[b]     = x[b] @ W[e_b] + b[e_b]                     # [H,W,C_OUT]
    lb_loss    = KL(usage || uniform), usage from one-hot(e_b) counts

Sharding: data-parallel over batch, 8 samples per core, expert/gate
weights replicated (per spec sharding_hint).

Per-core on-device pipeline (per sample):
  1. DMA x quarter [1024,256] -> SBUF natural layout [128pix, 8blk*256c]
  2. PE transpose each [128pix,128c] block against an augmented identity
     [128,129] whose last column is ones: one op yields both the x^T
     block AND the per-channel partial sum over its 128 pixels (pooling).
  3. Gate: reduce partials -> pooled, tiny fp32 PE matmul -> logits,
     argmax via reduce_max + is_ge, index -> DVE register, dynamic-slice
     (register offset) copies select W[e], b[e] from SBUF-resident weights.
  4. Main GEMM in float32r (full-rate fp32 matmul mode, N=256):
     out^ = x^T_blk.T @ W[e] accumulated over 2 K-chunks in PSUM; bias is
     fused into the mandatory PSUM->SBUF copy as a tensor_tensor add.
  5. Contiguous DMA of out quarters back to HBM.

lb_loss is finalized on host from the device-emitted one-hot routing
matrix (64x8) -- a trivial scalar reduction.
"""

import os

import numpy as np

from concourse import bacc, bass, mybir, tile
from concourse.bass_utils import run_bass_kernel_spmd

DT = mybir.dt.float32

B, HH, WW, CIN, COUT, E = 64, 64, 64, 256, 256, 8
PIX = HH * WW            # 4096 pixels per sample
NCORES = 8
S = B // NCORES          # 8 samples per core
NQ = int(os.environ.get("KERNEL_NQ", "4"))   # DMA chunks per sample
BPQ = 32 // NQ           # [128,256] pixel-blocks per chunk
NB = NQ * BPQ            # 32 pixel-blocks per sample
KC = CIN // 128          # 2 contraction chunks of 128
TW = 128                 # transpose block width (pixels)

_AX = mybir.AxisListType
_OP = mybir.AluOpType


def build(n_samples=S, mm_dt=None, tr_dt=None):
    """Build the per-core Bass program. Returns compiled Bacc instance."""
    mm_dt = mm_dt or os.environ.get("KERNEL_MM_DT", "f32r")
    MM = {"f32r": mybir.dt.float32r, "f32": mybir.dt.float32,
          "bf16": mybir.dt.bfloat16}[mm_dt]
    XD = {"f32r": mybir.dt.float32r, "f32": mybir.dt.float32,
          "bf16": mybir.dt.bfloat16}[mm_dt]  # on-chip x-path dtype
    # DRAM x dtype: f32r reuses the host-pre-rounded f32 bits; bf16 keeps f32
    # in HBM and casts during the SWDGE DMA.
    XSD = mybir.dt.float32r if mm_dt == "f32r" else mybir.dt.float32

    nc = bacc.Bacc("TRN2", target_bir_lowering=False, debug=False)

    xs = nc.dram_tensor("xs", (n_samples, PIX, CIN), XSD, kind="ExternalInput")
    wk = nc.dram_tensor("wk", (KC, 128, E * COUT), DT, kind="ExternalInput")
    bl = nc.dram_tensor("ball", (1, E * 2 * COUT), DT, kind="ExternalInput")
    gw = nc.dram_tensor("gw", (KC, 128, E), DT, kind="ExternalInput")
    gb = nc.dram_tensor("gb", (1, E), DT, kind="ExternalInput")
    idn = nc.dram_tensor("ident", (128, 128), XD, kind="ExternalInput")
    iot = nc.dram_tensor("iota", (1, E), DT, kind="ExternalInput")
    out_d = nc.dram_tensor("out", (n_samples, PIX, COUT), DT, kind="ExternalOutput")
    route_d = nc.dram_tensor("route", (1, n_samples * E), DT, kind="ExternalOutput")

    xs_ap, out_ap = xs.ap(), out_d.ap()

    xn_bufs = 8 if mm_dt == "bf16" else 3
    out_bufs = (4 if mm_dt == "bf16" else 2) if NQ == 4 else 2
    po_bufs = 4
    with tile.TileContext(nc) as tc:
        with (
            tc.tile_pool(name="const", bufs=1) as cp,
            tc.tile_pool(name="xn", bufs=xn_bufs) as xnp,
            tc.tile_pool(name="xt", bufs=2) as xtp,
            tc.tile_pool(name="wsel", bufs=2) as wsp,
            tc.tile_pool(name="outp", bufs=out_bufs) as op_,
            tc.tile_pool(name="small", bufs=2) as sp,
            tc.tile_pool(name="pt", bufs=3, space="PSUM") as ptp,
            tc.tile_pool(name="po", bufs=po_bufs, space="PSUM") as pop,
            tc.tile_pool(name="pg", bufs=1, space="PSUM") as pgp,
        ):
            # ---- resident constants ----
            wk_sb = []
            for k in range(KC):
                t = cp.tile([128, E * COUT], DT, tag=f"wk{k}")
                nc.sync.dma_start(t[:], wk.ap()[k])
                wk_sb.append(t)
            ball_sb = cp.tile([1, E * 2 * COUT], DT, tag="ball")
            nc.sync.dma_start(ball_sb[:], bl.ap())
            brep_all = cp.tile([128, E * 2 * COUT], DT, tag="brep_all")
            nc.gpsimd.partition_broadcast(brep_all[:], ball_sb[:])
            gw_sb = []
            for k in range(KC):
                t = cp.tile([128, E], DT, tag=f"gw{k}", name=f"gw{k}")
                nc.sync.dma_start(t[:], gw.ap()[k])
                gw_sb.append(t)
            gb_sb = cp.tile([1, E], DT, tag="gb")
            nc.sync.dma_start(gb_sb[:], gb.ap())
            id_sb = cp.tile([128, 128], XD, tag="ident")
            nc.sync.dma_start(id_sb[:], idn.ap())
            iota_sb = cp.tile([1, E], DT, tag="iota")
            nc.sync.dma_start(iota_sb[:], iot.ap())
            route_sb = cp.tile([1, n_samples * E], DT, tag="route")

            copy_split = os.environ.get("KERNEL_COPY_SPLIT", "1") == "1"
            pipelined = os.environ.get("KERNEL_PIPE", "1") == "1"

            def emit_transposes(s):
                """DMA-in + PE transposes + PSUM->SBUF copies (with pooling accum)."""
                xts = [xtp.tile([128, NB, TW], MM, tag=f"xt{k}", name=f"xt{k}_{s}") for k in range(KC)]
                parts = [sp.tile([128, NB], DT, tag=f"part{k}", name=f"part{k}_{s}") for k in range(KC)]
                for q in range(NQ):
                    xn = xnp.tile([128, BPQ, CIN], XD, tag="xn", name=f"xn_{s}_{q}")
                    src = xs_ap[s].rearrange("(p j) c -> p j c", j=32)[
                        :, q * BPQ : (q + 1) * BPQ, :
                    ]
                    if XD == mybir.dt.bfloat16:
                        nc.gpsimd.dma_start(xn[:], src)  # SWDGE cast f32->bf16
                    else:
                        nc.sync.dma_start(xn[:], src)
                    for b in range(BPQ):
                        blk = q * BPQ + b
                        for k in range(KC):
                            pt = ptp.tile([128, TW], XD, tag="pt", name=f"pt_{s}_{blk}_{k}")
                            nc.tensor.transpose(
                                pt[:],
                                xn[:, b, k * 128 : (k + 1) * 128],
                                id_sb[:],
                            )
                            if k == 0 or not copy_split:
                                nc.scalar.activation(
                                    xts[k][:, blk, :], pt[:],
                                    mybir.ActivationFunctionType.Copy,
                                    accum_out=parts[k][:, blk : blk + 1],
                                )
                            else:
                                nc.vector.tensor_scalar(
                                    xts[k][:, blk, :], pt[:], 1.0, None,
                                    _OP.mult, _OP.add,
                                    accum_out=parts[k][:, blk : blk + 1],
                                )
                return xts, parts

            def emit_gate(s, parts):
                """pooled -> logits -> argmax -> dynamic W[e]/b[e] select."""
                pooled = sp.tile([128, KC], DT, tag="pooled", name=f"pooled_{s}")
                for k in range(KC):
                    nc.vector.reduce_sum(pooled[:, k : k + 1], parts[k][:], axis=_AX.X)
                pg_t = pgp.tile([1, E], DT, tag="pg", name=f"pg_{s}")
                for k in range(KC):
                    nc.tensor.matmul(
                        pg_t[:], pooled[:, k : k + 1], gw_sb[k][:],
                        start=(k == 0), stop=(k == KC - 1),
                    )
                logits = sp.tile([1, E], DT, tag="logits", name=f"logits_{s}")
                nc.vector.scalar_tensor_tensor(
                    logits[:], pg_t[:], 1.0 / PIX, gb_sb[:], _OP.mult, _OP.add
                )
                maxv = sp.tile([1, 1], DT, tag="maxv", name=f"maxv_{s}")
                nc.vector.reduce_max(maxv[:], logits[:], axis=_AX.X)
                oh = sp.tile([1, E], DT, tag="oh", name=f"oh_{s}")
                nc.vector.tensor_scalar(oh[:], logits[:], maxv[:, 0:1], None, _OP.is_ge)
                nc.vector.tensor_copy(route_sb[:, s * E : (s + 1) * E], oh[:])
                ehit = sp.tile([1, E], DT, tag="ehit", name=f"ehit_{s}")
                nc.vector.tensor_tensor(ehit[:], oh[:], iota_sb[:], op=_OP.mult)
                idxf = sp.tile([1, 1], DT, tag="idxf", name=f"idxf_{s}")
                nc.vector.reduce_max(idxf[:], ehit[:], axis=_AX.X)
                idxi = sp.tile([1, 1], mybir.dt.int32, tag="idxi", name=f"idxi_{s}")
                nc.vector.tensor_copy(idxi[:], idxf[:])
                reg = nc.alloc_register(mybir.EngineType.DVE, f"eoff{s}")
                nc.vector.reg_load(reg, idxi[0:1, 0:1])
                off = nc.snap(reg, min_val=0, max_val=E - 1)

                weffs = []
                for k in range(KC):
                    t = wsp.tile([128, COUT], MM, tag=f"weff{k}", name=f"weff{k}_{s}")
                    wk3 = wk_sb[k][:].rearrange("p (e c) -> p e c", e=E)
                    nc.vector.tensor_copy(t[:], wk3[:, bass.ds(off, 1), :])
                    weffs.append(t)
                brep = brep_all[:].rearrange("p (e c) -> p e c", e=E)[:, bass.ds(off, 1), :]
                return weffs, brep

            def emit_mm(s, xts, weffs, brep):
                """main GEMM + fused bias add + DMA out."""
                for q in range(NQ):
                    osb = op_.tile([128, BPQ, COUT], DT, tag="osb", name=f"osb_{s}_{q}")
                    for b2 in range(BPQ // 2):
                        po = pop.tile([128, 2, COUT], DT, tag="po", name=f"po_{s}_{q}_{b2}")
                        for h in range(2):
                            blk = q * BPQ + b2 * 2 + h
                            for k in range(KC):
                                nc.tensor.matmul(
                                    po[:, h, :],
                                    xts[k][:, blk, :],
                                    weffs[k][:],
                                    start=(k == 0), stop=(k == KC - 1),
                                )
                        nc.vector.tensor_tensor(
                            osb[:, b2 * 2 : b2 * 2 + 2, :], po[:], brep, op=_OP.add
                        )
                    dst = out_ap[s].rearrange("(p j) c -> p j c", j=32)[
                        :, q * BPQ : (q + 1) * BPQ, :
                    ]
                    nc.sync.dma_start(dst, osb[:])

            if pipelined:
                prev = None  # (s, xts, weffs, brep)
                for i in range(n_samples + 1):
                    cur = None
                    if i < n_samples:
                        xts, parts = emit_transposes(i)
                    if prev is not None:
                        emit_mm(*prev)
                    if i < n_samples:
                        weffs, brep = emit_gate(i, parts)
                        cur = (i, xts, weffs, brep)
                    prev = cur
            else:
                for i in range(n_samples):
                    xts, parts = emit_transposes(i)
                    weffs, brep = emit_gate(i, parts)
                    emit_mm(i, xts, weffs, brep)

            nc.sync.dma_start(route_d.ap(), route_sb[:])

    nc.compile()
    return nc


def make_host_inputs(W, b, gate_W, gate_b, mm_dt=None):
    """Host-side constant marshaling (replicated across cores)."""
    mm_dt = mm_dt or os.environ.get("KERNEL_MM_DT", "f32r")
    W = np.asarray(W, dtype=np.float32)
    wk = np.ascontiguousarray(
        W.reshape(E, KC, 128, COUT).transpose(1, 2, 0, 3).reshape(KC, 128, E * COUT)
    )
    b2 = np.asarray(b, np.float32).reshape(E, 1, COUT)
    ball = np.ascontiguousarray(np.broadcast_to(b2, (E, 2, COUT)).reshape(1, E * 2 * COUT))
    gw = np.ascontiguousarray(np.asarray(gate_W, np.float32).reshape(KC, 128, E))
    gbv = np.ascontiguousarray(np.asarray(gate_b, np.float32).reshape(1, E))
    if mm_dt == "bf16":
        import ml_dtypes
        ident = np.eye(128, dtype=ml_dtypes.bfloat16)
    else:
        ident = np.eye(128, dtype=np.float32)  # exact in f32 and f32r
    iota = np.arange(E, dtype=np.float32).reshape(1, E)
    return dict(wk=wk, ball=ball, gw=gw, gb=gbv, ident=ident, iota=iota)


_BUILT = None
last_results = None
_RESET_DONE = False


def _axon_reset():
    """Clear any wedged device state left by a previous crashed process."""
    global _RESET_DONE
    if _RESET_DONE:
        return
    _RESET_DONE = True
    try:
        import ctypes

        import jax

        jax.devices()
        lib = ctypes.CDLL("/opt/axon/libaxon_pjrt.so")
        lib.axon_reset.restype = ctypes.c_int64
        lib.axon_reset()
    except Exception:
        pass


def _round_fp32r(x):
    """RNE to fp32r (11 explicit mantissa bits); matches neuron_dtypes."""
    b = np.ascontiguousarray(x, dtype=np.float32).view(np.uint32)
    add = np.uint32(0x7FF) + ((b >> np.uint32(12)) & np.uint32(1))
    return ((b + add) & np.uint32(0xFFFFF000)).view(np.float32)


def kernel(x, W, b, gate_W, gate_b):
    global _BUILT, last_results
    x = np.ascontiguousarray(np.asarray(x, dtype=np.float32))
    if os.environ.get("KERNEL_MM_DT", "f32r") == "f32r":
        x = _round_fp32r(x)
    consts = make_host_inputs(W, b, gate_W, gate_b)
    _axon_reset()
    if _BUILT is None:
        _BUILT = build()
    nc = _BUILT

    xr = x.reshape(B, PIX, CIN)
    in_maps = [
        {"xs": np.ascontiguousarray(xr[c * S : (c + 1) * S]), **consts}
        for c in range(NCORES)
    ]
    res = run_bass_kernel_spmd(nc, in_maps, core_ids=list(range(NCORES)))
    last_results = res

    out = np.empty((B, PIX, COUT), np.float32)
    for c in range(NCORES):
        out[c * S : (c + 1) * S] = res.results[c]["out"]
    out = out.reshape(B, HH, WW, COUT)

    oneh = np.concatenate(
        [res.results[c]["route"].reshape(S, E) for c in range(NCORES)], axis=0
    ).astype(np.float32)
    usage = oneh.mean(axis=0) + np.float32(1e-6)
    usage = usage / usage.sum()
    uniform = np.full(E, 1.0 / E, np.float32)
    lb = np.sum(usage * (np.log(usage) - np.log(uniform))).astype(np.float32)
    return out, lb


# revision 29
# speedup vs baseline: 1.2497x; 1.2497x over previous
"""MoE dense kernel (nn_MoEDense) for 8 Trainium2 NeuronCores.

Reference computation (per batch sample b):
    pooled[b]  = mean_{h,w} x[b,h,w,:]                      # [C_IN]
    logits[b]  = pooled[b] @ gate_W + gate_b                # [E]
    e_b        = argmax(logits[b])       (softmax is monotonic -> skip it)
    out[b]     = x[b] @ W[e_b] + b[e_b]                     # [H,W,C_OUT]
    lb_loss    = KL(usage || uniform), usage from one-hot(e_b) counts

Sharding: data-parallel over batch — 8 samples per core, expert/gate
weights replicated (spec sharding_hint). Each core runs the same Bass
program via run_bass_kernel_spmd on cores 0-7.

Math dtype (KERNEL_MM_DT): default "f32r" — the PE's fast fp32 mode
(fp32 RNE-rounded to 11 explicit mantissa bits, streamed at 2 cyc/row
vs 4 for plain fp32). x is pre-rounded on the host so the whole x path
is legal fp32r (the BIR verifier requires fp32r matmul inputs to be
produced rounded). End-to-end rel err ~1.4e-4. "bf16" (~19% faster,
rel err ~2.6e-3) and exact "f32" are also implemented.

Per-core pipeline, software-pipelined across samples
(transposes(i) -> main-GEMM(i-1) -> gate(i)):
  1. DMA x in 1MB quarters -> SBUF [128 pix, 8 blk, 256 c]; the
     pixel->partition mapping (pixel = 32*p + j) keeps every DMA
     8KB-contiguous per partition on both input and output.
  2. PE transpose-mode matmuls flip each [128pix,128c] block into
     x^T form (contraction dim c must sit on partitions for the GEMM);
     the mandatory PSUM->SBUF copy rides ACT (k=0) / DVE (k=1) with
     accum_out producing the per-block pooling partials for free.
  3. Gate: DVE-reduce partials -> pooled, tiny fp32 PE matmul ->
     logits, argmax via reduce_max + is_ge, expert index loaded into a
     DVE register; W[e] selected by register-offset dynamic slices
     (bass.ds) out of SBUF-resident weights. One-hot rows are emitted
     to the "route" output for host-side lb_loss.
  4. Main GEMM: out[pix,c_out] = xT_blk.T @ W[e], 2 K-chunks
     accumulated per PSUM bank, two pixel-blocks paired per bank; the
     bias add is fused into the PSUM->SBUF copy (tensor_tensor add
     against a pre-replicated all-expert bias table, selected by the
     same register).
  5. Contiguous 1MB DMA of out quarters back to HBM.

lb_loss is finalized on host from the device-emitted one-hot routing
matrix (64x8) — a trivial 512-element reduction done in fp32 exactly
as the reference does it.

Measured on 8 axon-tunneled trn2 cores (full 64-sample problem):
~277 us HW exec (f32r), ~233 us (bf16); HBM roofline for the 64 MB
per-core traffic is ~179 us.
"""

import os

import numpy as np

from concourse import bacc, bass, mybir, tile
from concourse.bass_utils import run_bass_kernel_spmd

DT = mybir.dt.float32

B, HH, WW, CIN, COUT, E = 64, 64, 64, 256, 256, 8
PIX = HH * WW            # 4096 pixels per sample
NCORES = 8
S = B // NCORES          # 8 samples per core
NQ = int(os.environ.get("KERNEL_NQ", "4"))   # DMA chunks per sample
BPQ = 32 // NQ           # [128,256] pixel-blocks per chunk
NB = NQ * BPQ            # 32 pixel-blocks per sample
KC = CIN // 128          # 2 contraction chunks of 128
TW = 128                 # transpose block width (pixels)

_AX = mybir.AxisListType
_OP = mybir.AluOpType


def build(n_samples=S, mm_dt=None):
    """Build the per-core Bass program. Returns compiled Bacc instance."""
    mm_dt = mm_dt or os.environ.get("KERNEL_MM_DT", "f32r")
    MM = {"f32r": mybir.dt.float32r, "f32": mybir.dt.float32,
          "bf16": mybir.dt.bfloat16}[mm_dt]
    XD = {"f32r": mybir.dt.float32r, "f32": mybir.dt.float32,
          "bf16": mybir.dt.bfloat16}[mm_dt]  # on-chip x-path dtype
    # DRAM x dtype: f32r reuses the host-pre-rounded f32 bits; bf16 keeps f32
    # in HBM and casts during the SWDGE DMA.
    XSD = mybir.dt.float32r if mm_dt == "f32r" else mybir.dt.float32

    nc = bacc.Bacc("TRN2", target_bir_lowering=False, debug=False)

    xs = nc.dram_tensor("xs", (n_samples, PIX, CIN), XSD, kind="ExternalInput")
    wk = nc.dram_tensor("wk", (KC, 128, E * COUT), DT, kind="ExternalInput")
    bl = nc.dram_tensor("ball", (1, E * 2 * COUT), DT, kind="ExternalInput")
    gw = nc.dram_tensor("gw", (KC, 128, E), DT, kind="ExternalInput")
    gb = nc.dram_tensor("gb", (1, E), DT, kind="ExternalInput")
    idn = nc.dram_tensor("ident", (128, 128), XD, kind="ExternalInput")
    iot = nc.dram_tensor("iota", (1, E), DT, kind="ExternalInput")
    out_d = nc.dram_tensor("out", (n_samples, PIX, COUT), DT, kind="ExternalOutput")
    route_d = nc.dram_tensor("route", (1, n_samples * E), DT, kind="ExternalOutput")

    xs_ap, out_ap = xs.ap(), out_d.ap()

    xn_bufs = 8 if mm_dt == "bf16" else 3
    out_bufs = (4 if mm_dt == "bf16" else 2) if NQ == 4 else 2
    po_bufs = int(os.environ.get("KERNEL_PO", "4"))
    with tile.TileContext(nc) as tc:
        with (
            tc.tile_pool(name="const", bufs=1) as cp,
            tc.tile_pool(name="xn", bufs=xn_bufs) as xnp,
            tc.tile_pool(name="xt", bufs=2) as xtp,
            tc.tile_pool(name="wsel", bufs=2) as wsp,
            tc.tile_pool(name="outp", bufs=out_bufs) as op_,
            tc.tile_pool(name="small", bufs=2) as sp,
            tc.tile_pool(name="pt", bufs=int(os.environ.get("KERNEL_PT", "3")), space="PSUM") as ptp,
            tc.tile_pool(name="po", bufs=po_bufs, space="PSUM") as pop,
            tc.tile_pool(name="pg", bufs=1, space="PSUM") as pgp,
        ):
            # ---- resident constants ----
            wk_sb = []
            for k in range(KC):
                t = cp.tile([128, E * COUT], DT, tag=f"wk{k}")
                nc.sync.dma_start(t[:], wk.ap()[k])
                wk_sb.append(t)
            ball_sb = cp.tile([1, E * 2 * COUT], DT, tag="ball")
            nc.sync.dma_start(ball_sb[:], bl.ap())
            brep_all = cp.tile([128, E * 2 * COUT], DT, tag="brep_all")
            nc.gpsimd.partition_broadcast(brep_all[:], ball_sb[:])
            gw_sb = []
            for k in range(KC):
                t = cp.tile([128, E], DT, tag=f"gw{k}", name=f"gw{k}")
                nc.sync.dma_start(t[:], gw.ap()[k])
                gw_sb.append(t)
            gb_sb = cp.tile([1, E], DT, tag="gb")
            nc.sync.dma_start(gb_sb[:], gb.ap())
            id_sb = cp.tile([128, 128], XD, tag="ident")
            nc.sync.dma_start(id_sb[:], idn.ap())
            iota_sb = cp.tile([1, E], DT, tag="iota")
            nc.sync.dma_start(iota_sb[:], iot.ap())
            route_sb = cp.tile([1, n_samples * E], DT, tag="route")

            copy_split = os.environ.get("KERNEL_COPY_SPLIT", "1") == "1"
            pipelined = os.environ.get("KERNEL_PIPE", "1") == "1"

            def emit_transposes(s):
                """DMA-in + PE transposes + PSUM->SBUF copies (with pooling accum)."""
                xts = [xtp.tile([128, NB, TW], MM, tag=f"xt{k}", name=f"xt{k}_{s}") for k in range(KC)]
                parts = [sp.tile([128, NB], DT, tag=f"part{k}", name=f"part{k}_{s}") for k in range(KC)]
                for q in range(NQ):
                    xn = xnp.tile([128, BPQ, CIN], XD, tag="xn", name=f"xn_{s}_{q}")
                    src = xs_ap[s].rearrange("(p j) c -> p j c", j=32)[
                        :, q * BPQ : (q + 1) * BPQ, :
                    ]
                    if XD == mybir.dt.bfloat16:
                        nc.gpsimd.dma_start(xn[:], src)  # SWDGE cast f32->bf16
                    else:
                        nc.sync.dma_start(xn[:], src)
                    for b in range(BPQ):
                        blk = q * BPQ + b
                        for k in range(KC):
                            pt = ptp.tile([128, TW], XD, tag="pt", name=f"pt_{s}_{blk}_{k}")
                            nc.tensor.transpose(
                                pt[:],
                                xn[:, b, k * 128 : (k + 1) * 128],
                                id_sb[:],
                            )
                            if k == 0 or not copy_split:
                                nc.scalar.activation(
                                    xts[k][:, blk, :], pt[:],
                                    mybir.ActivationFunctionType.Copy,
                                    accum_out=parts[k][:, blk : blk + 1],
                                )
                            else:
                                nc.vector.tensor_scalar(
                                    xts[k][:, blk, :], pt[:], 1.0, None,
                                    _OP.mult, _OP.add,
                                    accum_out=parts[k][:, blk : blk + 1],
                                )
                return xts, parts

            def emit_gate(s, parts):
                """pooled -> logits -> argmax -> dynamic W[e]/b[e] select."""
                pooled = sp.tile([128, KC], DT, tag="pooled", name=f"pooled_{s}")
                for k in range(KC):
                    nc.vector.reduce_sum(pooled[:, k : k + 1], parts[k][:], axis=_AX.X)
                pg_t = pgp.tile([1, E], DT, tag="pg", name=f"pg_{s}")
                for k in range(KC):
                    nc.tensor.matmul(
                        pg_t[:], pooled[:, k : k + 1], gw_sb[k][:],
                        start=(k == 0), stop=(k == KC - 1),
                    )
                logits = sp.tile([1, E], DT, tag="logits", name=f"logits_{s}")
                nc.vector.scalar_tensor_tensor(
                    logits[:], pg_t[:], 1.0 / PIX, gb_sb[:], _OP.mult, _OP.add
                )
                maxv = sp.tile([1, 1], DT, tag="maxv", name=f"maxv_{s}")
                nc.vector.reduce_max(maxv[:], logits[:], axis=_AX.X)
                oh = sp.tile([1, E], DT, tag="oh", name=f"oh_{s}")
                nc.vector.tensor_scalar(oh[:], logits[:], maxv[:, 0:1], None, _OP.is_ge)
                nc.vector.tensor_copy(route_sb[:, s * E : (s + 1) * E], oh[:])
                ehit = sp.tile([1, E], DT, tag="ehit", name=f"ehit_{s}")
                nc.vector.tensor_tensor(ehit[:], oh[:], iota_sb[:], op=_OP.mult)
                idxf = sp.tile([1, 1], DT, tag="idxf", name=f"idxf_{s}")
                nc.vector.reduce_max(idxf[:], ehit[:], axis=_AX.X)
                idxi = sp.tile([1, 1], mybir.dt.int32, tag="idxi", name=f"idxi_{s}")
                nc.vector.tensor_copy(idxi[:], idxf[:])
                reg = nc.alloc_register(mybir.EngineType.DVE, f"eoff{s}")
                nc.vector.reg_load(reg, idxi[0:1, 0:1])
                off = nc.snap(reg, min_val=0, max_val=E - 1)

                weffs = []
                for k in range(KC):
                    t = wsp.tile([128, COUT], MM, tag=f"weff{k}", name=f"weff{k}_{s}")
                    wk3 = wk_sb[k][:].rearrange("p (e c) -> p e c", e=E)
                    nc.vector.tensor_copy(t[:], wk3[:, bass.ds(off, 1), :])
                    weffs.append(t)
                brep = brep_all[:].rearrange("p (e c) -> p e c", e=E)[:, bass.ds(off, 1), :]
                return weffs, brep

            def emit_mm(s, xts, weffs, brep):
                """main GEMM + fused bias add + DMA out."""
                for q in range(NQ):
                    osb = op_.tile([128, BPQ, COUT], DT, tag="osb", name=f"osb_{s}_{q}")
                    for b2 in range(BPQ // 2):
                        po = pop.tile([128, 2, COUT], DT, tag="po", name=f"po_{s}_{q}_{b2}")
                        for h in range(2):
                            blk = q * BPQ + b2 * 2 + h
                            for k in range(KC):
                                nc.tensor.matmul(
                                    po[:, h, :],
                                    xts[k][:, blk, :],
                                    weffs[k][:],
                                    start=(k == 0), stop=(k == KC - 1),
                                )
                        nc.vector.tensor_tensor(
                            osb[:, b2 * 2 : b2 * 2 + 2, :], po[:], brep, op=_OP.add
                        )
                    dst = out_ap[s].rearrange("(p j) c -> p j c", j=32)[
                        :, q * BPQ : (q + 1) * BPQ, :
                    ]
                    nc.sync.dma_start(dst, osb[:])

            if pipelined:
                prev = None  # (s, xts, weffs, brep)
                for i in range(n_samples + 1):
                    cur = None
                    if i < n_samples:
                        xts, parts = emit_transposes(i)
                    if prev is not None:
                        emit_mm(*prev)
                    if i < n_samples:
                        weffs, brep = emit_gate(i, parts)
                        cur = (i, xts, weffs, brep)
                    prev = cur
            else:
                for i in range(n_samples):
                    xts, parts = emit_transposes(i)
                    weffs, brep = emit_gate(i, parts)
                    emit_mm(i, xts, weffs, brep)

            nc.sync.dma_start(route_d.ap(), route_sb[:])

    nc.compile()
    return nc


def make_host_inputs(W, b, gate_W, gate_b, mm_dt=None):
    """Host-side constant marshaling (replicated across cores)."""
    mm_dt = mm_dt or os.environ.get("KERNEL_MM_DT", "f32r")
    W = np.asarray(W, dtype=np.float32)
    wk = np.ascontiguousarray(
        W.reshape(E, KC, 128, COUT).transpose(1, 2, 0, 3).reshape(KC, 128, E * COUT)
    )
    b2 = np.asarray(b, np.float32).reshape(E, 1, COUT)
    ball = np.ascontiguousarray(np.broadcast_to(b2, (E, 2, COUT)).reshape(1, E * 2 * COUT))
    gw = np.ascontiguousarray(np.asarray(gate_W, np.float32).reshape(KC, 128, E))
    gbv = np.ascontiguousarray(np.asarray(gate_b, np.float32).reshape(1, E))
    if mm_dt == "bf16":
        import ml_dtypes
        ident = np.eye(128, dtype=ml_dtypes.bfloat16)
    else:
        ident = np.eye(128, dtype=np.float32)  # exact in f32 and f32r
    iota = np.arange(E, dtype=np.float32).reshape(1, E)
    return dict(wk=wk, ball=ball, gw=gw, gb=gbv, ident=ident, iota=iota)


_BUILT = None
last_results = None
_RESET_DONE = False


def _axon_reset():
    """Clear any wedged device state left by a previous crashed process."""
    global _RESET_DONE
    if _RESET_DONE:
        return
    _RESET_DONE = True
    try:
        import ctypes

        import jax

        jax.devices()
        lib = ctypes.CDLL("/opt/axon/libaxon_pjrt.so")
        lib.axon_reset.restype = ctypes.c_int64
        lib.axon_reset()
    except Exception:
        pass


def _round_fp32r(x):
    """RNE to fp32r (11 explicit mantissa bits); matches neuron_dtypes."""
    b = np.ascontiguousarray(x, dtype=np.float32).view(np.uint32)
    add = np.uint32(0x7FF) + ((b >> np.uint32(12)) & np.uint32(1))
    return ((b + add) & np.uint32(0xFFFFF000)).view(np.float32)


def kernel(x, W, b, gate_W, gate_b):
    global _BUILT, last_results
    x = np.ascontiguousarray(np.asarray(x, dtype=np.float32))
    if os.environ.get("KERNEL_MM_DT", "f32r") == "f32r":
        x = _round_fp32r(x)
    consts = make_host_inputs(W, b, gate_W, gate_b)
    _axon_reset()
    if _BUILT is None:
        _BUILT = build()
    nc = _BUILT

    xr = x.reshape(B, PIX, CIN)
    in_maps = [
        {"xs": np.ascontiguousarray(xr[c * S : (c + 1) * S]), **consts}
        for c in range(NCORES)
    ]
    res = run_bass_kernel_spmd(nc, in_maps, core_ids=list(range(NCORES)))
    last_results = res

    out = np.empty((B, PIX, COUT), np.float32)
    for c in range(NCORES):
        out[c * S : (c + 1) * S] = res.results[c]["out"]
    out = out.reshape(B, HH, WW, COUT)

    oneh = np.concatenate(
        [res.results[c]["route"].reshape(S, E) for c in range(NCORES)], axis=0
    ).astype(np.float32)
    usage = oneh.mean(axis=0) + np.float32(1e-6)
    usage = usage / usage.sum()
    uniform = np.full(E, 1.0 / E, np.float32)
    lb = np.sum(usage * (np.log(usage) - np.log(uniform))).astype(np.float32)
    return out, lb
